# revision 56
# baseline (speedup 1.0000x reference)
"""Multi-head attention (RoPE) Trainium2 Bass kernel — pipelined bf16 version.

Problem: B=4, T=2048, C=1024, H=16, d=64, fp32 in/out, full attention + RoPE.
Sharding: 8 cores = 4 batches x 2 head-groups (8 heads each). Each core
computes its batch's attention for its heads plus the partial (transposed)
output projection; the host sums the two head-group partials per batch and
transposes back.

Design notes (cost-model driven):
- All matmul operands are bf16 (1 cycle/row on PE, half the SBUF/DMA of f32).
- AV uses a token-major dataflow: out[q, 65] = ex_chunk^T @ [ones|v], using
  all 128 output partitions (halves AV PE time vs a 65-partition head-major
  form) and making softmax normalization a per-partition scalar multiply.
  The softmax denominator rides along as column 0 via the ones column of vg.
- Normalized attention output transposes back to head-dim-major via one
  SBUF->SBUF DMA xbar transpose per (head-pair, q-half); PE is not involved.
- One instruction stream software-pipelines everything: QKV chunk
  projections, v-projections and the second-half output projection run as PE
  filler inside the ACT-bound attention stretch so neither PE nor the
  Activation engine (exp) ever starves. RoPE for chunk i-1 is emitted inside
  chunk i's slot so its PE permutation-matmul never waits on DVE.
- Projection is emitted transposed (features on partitions) so its bias is a
  per-partition scalar; the host transposes the final result (untimed).
"""

import numpy as np
import ml_dtypes

B, T, C = 4, 2048, 1024
H, D = 16, 64
G = 2              # head groups (cores per batch)
HG = H // G        # heads per core = 8
CC = C // 128      # 8 contraction chunks
NKC = T // 128     # 16 key chunks
NTB = T // 512     # 4 t-blocks
ROPE_BASE = 10000.0
SCALE = 1.0 / np.sqrt(D)

FUSED_NORM = True      # stride-0 free-dim broadcast of 1/den in one DVE op
TRANSPOSE_3D = True    # one xbar DMA transpose per (pair, q-half)

_CACHED = {}


def _rope_tables():
    inv_freq = 1.0 / (ROPE_BASE ** (np.arange(0, D, 2, dtype=np.float32) / D))
    t = np.arange(T, dtype=np.float32)
    freqs = np.outer(t, inv_freq).astype(np.float32)          # (T, 32)
    emb = np.concatenate([freqs, freqs], axis=-1)             # (T, 64)
    cos = np.cos(emb).T.astype(np.float32)                    # (64, T)
    sin = np.sin(emb).T.astype(np.float32)                    # (64, T)
    cosT = np.concatenate([cos, cos], axis=0)                 # (128, T) two heads/chunk
    sinT = np.concatenate([sin, sin], axis=0)
    return np.ascontiguousarray(cosT), np.ascontiguousarray(sinT)


def _perm_table():
    # rot[d] = sum_s P[s, d] * raw[s] = rotate_half with sign, 2 heads/chunk
    P = np.zeros((128, 128), np.float32)
    for d in range(128):
        blk, dd = divmod(d, D)
        if dd < 32:
            P[blk * D + dd + 32, d] = -1.0
        else:
            P[blk * D + dd - 32, d] = 1.0
    return P


def _attn_body(tc, outs, ins):
    """Tile kernel body. ins/outs are dicts of DRAM APs."""
    import contextlib
    import concourse.bass as bass
    import concourse.mybir as mybir

    nc = tc.nc
    F32 = mybir.dt.float32
    BF16 = mybir.dt.bfloat16
    EXP = mybir.ActivationFunctionType.Exp

    xT = ins["xT"]            # (1024, 2048) bf16  x[b].T
    wqkv = ins["wqkv"]        # (1024, 1536) bf16  [Wq | Wk | Wv] cols for group
    wproj = ins["wproj"]      # (512, 1024) bf16
    bqk = ins["bqk"]          # (128, 8) f32 per-chunk per-partition bias
    bv = ins["bv"]            # (128, 520) f32 broadcast [1|v-bias] per head
    bpr = ins["bpr"]          # (128, 8) f32 proj bias (e-chunk cols; zeros g1)
    cosT_d = ins["cosT"]      # (128, 2048) bf16
    sinT_d = ins["sinT"]      # (128, 2048) bf16
    perm_d = ins["rope_perm"]  # (128, 128) bf16 signed rotate_half permutation
    out = outs["out"]         # (1024, 2048) f32 partial transposed output

    def dbg(name, tile_ap):
        if name in outs:
            nc.sync.dma_start(outs[name].bitcast(tile_ap.dtype), tile_ap)

    ctx = contextlib.ExitStack()
    with ctx:
        pers = ctx.enter_context(tc.tile_pool(name="pers", bufs=1))

        # ---------------- persistent tiles ----------------
        x_t = pers.tile([128, CC * T], BF16, name="x_t", tag="x_t")
        wqk_t = pers.tile([128, CC * 1024], BF16, name="wqk_t", tag="wqk_t")
        wv_t = pers.tile([128, CC * 512], BF16, name="wv_t", tag="wv_t")
        wp_t = pers.tile([128, 4 * 1024], BF16, name="wp_t", tag="wp_t")
        cos_t = pers.tile([128, T], BF16, name="cos_t", tag="cos_t")
        sin_t = pers.tile([128, T], BF16, name="sin_t", tag="sin_t")
        perm_t = pers.tile([128, 128], BF16, name="perm_t", tag="perm_t")
        bqk_t = pers.tile([128, 8], F32, name="bqk_t", tag="bqk_t")
        bv_t = pers.tile([128, 520], F32, name="bv_t", tag="bv_t")
        bpr_t = pers.tile([128, 8], F32, name="bpr_t", tag="bpr_t")
        qk = [pers.tile([128, T], BF16, name=f"qk{j}", tag=f"qk{j}") for j in range(8)]
        vg = [pers.tile([128, HG * 65], BF16, name=f"vg{k}", tag=f"vg{k}") for k in range(NKC)]
        aT = [pers.tile([128, T], BF16, name=f"aT{i}", tag=f"aT{i}") for i in range(4)]

        # ---------------- working pools ----------------
        # PSUM: psS 2x2 banks (scores/exp), psAV 1x2 banks (AV accum),
        # psF 1x2 banks (qkv/v/proj filler groups + rope perm outputs).
        psS = ctx.enter_context(tc.tile_pool(name="psS", bufs=2, space="PSUM"))
        psAV = ctx.enter_context(tc.tile_pool(name="psAV", bufs=1, space="PSUM"))
        psF = ctx.enter_context(tc.tile_pool(name="psF", bufs=1, space="PSUM"))
        expool = ctx.enter_context(tc.tile_pool(name="expool", bufs=6))
        ex1p = ctx.enter_context(tc.tile_pool(name="ex1p", bufs=16))
        rawp = ctx.enter_context(tc.tile_pool(name="rawp", bufs=2))
        tmpp = ctx.enter_context(tc.tile_pool(name="tmpp", bufs=2))
        tmpcp = ctx.enter_context(tc.tile_pool(name="tmpcp", bufs=2))
        denp = ctx.enter_context(tc.tile_pool(name="denp", bufs=2))
        rcpp = ctx.enter_context(tc.tile_pool(name="rcpp", bufs=2))
        avnp = ctx.enter_context(tc.tile_pool(name="avnp", bufs=2))
        osbp = ctx.enter_context(tc.tile_pool(name="osbp", bufs=5))

        # ---------------- input DMAs (no waits; ordered for earliest use) ---
        x3d = x_t.rearrange("p (c t) -> p c t", t=T)
        xTd = xT.rearrange("(c p) t -> p c t", p=128)
        wqk3 = wqk_t.rearrange("p (c e) -> p c e", e=1024)

        def w_slice(jc):
            # per-chunk 128-col slice of [Wq|Wk] for q/k chunk jc
            col0 = (jc % 4) * 128 + (512 if jc >= 4 else 0)
            nc.sync.dma_start(
                wqk3[:, :, col0:col0 + 128],
                wqkv[:, col0:col0 + 128].rearrange("(c p) e -> p c e", p=128))

        nc.sync.dma_start(perm_t, perm_d)
        w_slice(0)
        nc.sync.dma_start(x3d[:, :, 0:512], xTd[:, :, 0:512])
        nc.sync.dma_start(bqk_t, bqk)
        nc.sync.dma_start(cos_t[:, 0:1024], cosT_d[:, 0:1024])
        nc.sync.dma_start(sin_t[:, 0:1024], sinT_d[:, 0:1024])
        w_slice(4)
        nc.sync.dma_start(x3d[:, :, 512:1024], xTd[:, :, 512:1024])
        nc.sync.dma_start(cos_t[:, 1024:2048], cosT_d[:, 1024:2048])
        nc.sync.dma_start(sin_t[:, 1024:2048], sinT_d[:, 1024:2048])
        nc.sync.dma_start(wv_t.rearrange("p (c e) -> p c e", e=512),
                          wqkv[:, 1024:1536].rearrange("(c p) e -> p c e", p=128))
        nc.sync.dma_start(bv_t, bv)
        nc.sync.dma_start(x3d[:, :, 1024:1536], xTd[:, :, 1024:1536])
        nc.sync.dma_start(x3d[:, :, 1536:2048], xTd[:, :, 1536:2048])
        for jc in (1, 5, 2, 6, 3, 7):
            w_slice(jc)
        nc.sync.dma_start(bpr_t, bpr)
        nc.sync.dma_start(wp_t.rearrange("p (i e) -> p i e", e=1024),
                          wproj.rearrange("(i p) e -> p i e", p=128))

        # PE clock warmup: keep the tensor engine continuously busy from the
        # moment perm_t lands until the first real slot's inputs arrive, so
        # the p-state model reaches full clock before real work dispatches.
        warm = psF.tile([128, 1024], F32, name="warm", tag="s")
        for i in range(40):
            nc.tensor.matmul(warm[:, 0:128], perm_t, perm_t, start=True, stop=True)

        # ---------------- emitters ----------------
        pend = {"rope": None}  # (jc, tb, raw) awaiting perm-matmul + combine

        def emit_rope(ps_half):
            """Emit pending RoPE combine: perm-matmul into ps_half (psum
            (128,512) f32 slice), then DVE combine into qk[jc]."""
            jc, tb, raw = pend["rope"]
            pend["rope"] = None
            tsl = slice(tb * 512, (tb + 1) * 512)
            nc.tensor.matmul(ps_half, perm_t, raw, start=True, stop=True)
            tmp = tmpp.tile([128, 512], F32, name=f"tm{jc}_{tb}", tag="tmp")
            nc.vector.tensor_mul(tmp, ps_half, sin_t[:, tsl])
            tmpc = tmpcp.tile([128, 512], F32, name=f"tc{jc}_{tb}", tag="tmpc")
            nc.vector.tensor_mul(tmpc, raw, cos_t[:, tsl])
            nc.vector.tensor_add(qk[jc][:, tsl], tmp, tmpc)

        def emit_qk_part(jc, tb, pool, part, box):
            """Half of a q/k chunk slot: part 0 allocs the psum tile, emits
            the pending RoPE combine and contraction chunks 0-3; part 1 emits
            chunks 4-7 and the bias. Splitting keeps each filler burst under
            the exp pipeline's slack."""
            col0 = (jc % 4) * 128 + (512 if jc >= 4 else 0)
            if part == 0:
                ps = pool.tile([128, 1024], F32, name=f"psq{jc}_{tb}", tag="s")
                box["ps"] = ps
                if pend["rope"] is not None:
                    emit_rope(ps[:, 512:1024])
                crange = range(0, CC // 2)
            else:
                ps = box["ps"]
                crange = range(CC // 2, CC)
            for c in crange:
                nc.tensor.matmul(
                    ps[:, 0:512], wqk_t[:, c * 1024 + col0:c * 1024 + col0 + 128],
                    x_t[:, c * T + tb * 512:c * T + (tb + 1) * 512],
                    start=(c == 0), stop=(c == CC - 1))
            if part == 1:
                raw = rawp.tile([128, 512], BF16, name=f"raw{jc}_{tb}", tag="raw")
                nc.vector.tensor_scalar_add(raw, ps[:, 0:512], bqk_t[:, jc:jc + 1])
                pend["rope"] = (jc, tb, raw)

        def emit_qk_slot(jc, tb, pool):
            """8 projection matmuls for q/k chunk jc, t-block tb, plus the
            RoPE combine of the previously emitted chunk."""
            box = {}
            emit_qk_part(jc, tb, pool, 0, box)
            emit_qk_part(jc, tb, pool, 1, box)

        def emit_rope_flush(pool):
            ps = pool.tile([128, 1024], F32, name="psflush", tag="s")
            emit_rope(ps[:, 512:1024])

        def emit_v(kc, pool):
            """v for token chunk kc -> vg[kc] = [1|v] per head, bf16."""
            ps = pool.tile([128, 1024], F32, name=f"psv{kc}", tag="s")
            for c in range(CC):
                nc.tensor.matmul(
                    ps[:, 0:512], x_t[:, c * T + kc * 128:c * T + (kc + 1) * 128],
                    wv_t[:, c * 512:(c + 1) * 512],
                    start=(c == 0), stop=(c == CC - 1))
            vv = vg[kc].rearrange("p (g w) -> p g w", w=65)
            bvv = bv_t.rearrange("p (g w) -> p g w", w=65)
            psg = ps[:, 0:512].rearrange("p (g d) -> p g d", d=64)
            nc.vector.tensor_add(vv[:, :, 1:65], psg, bvv[:, :, 1:65])
            nc.vector.tensor_copy(vv[:, :, 0:1], bvv[:, :, 0:1])

        def emit_proj(ec, th, pool):
            """transposed proj: out rows = e-chunk ec, cols = tok block th."""
            tsl = slice(th * 512, (th + 1) * 512)
            ps = pool.tile([128, 1024], F32, name=f"psp{ec}_{th}", tag="s")
            for i in range(4):
                nc.tensor.matmul(
                    ps[:, 0:512], wp_t[:, i * 1024 + ec * 128:i * 1024 + (ec + 1) * 128],
                    aT[i][:, tsl], start=(i == 0), stop=(i == 3))
            osb = osbp.tile([128, 512], F32, name=f"osb{ec}_{th}", tag="osb")
            nc.vector.tensor_scalar_add(osb, ps[:, 0:512], bpr_t[:, ec:ec + 1])
            nc.sync.dma_start(out[ec * 128:(ec + 1) * 128, tsl], osb)

        def emit_av(pav, ex, kc, h):
            # start=True zeroes the whole PSUM bank, so only the first group
            # of each bank (qc 0 and 4) may set it; the bank-wide zero covers
            # the other interleaved accumulation groups' regions.
            mv = vg[kc][:, h * 65:(h + 1) * 65]
            for qc in range(8):
                nc.tensor.matmul(
                    pav[:, qc * 128:qc * 128 + 65],
                    ex[:, qc * 128:(qc + 1) * 128], mv,
                    start=(kc == 0 and qc % 4 == 0), stop=(kc == NKC - 1))

        def norm(pav, p, avn3, q0=0, q1=8):
            """normalize: avn[:, tc, p*64+d] = pav[:, tc, 1+d] / pav[:, tc, 0]"""
            ho = p * 64
            nq = q1 - q0
            pavr = pav.rearrange("p (qc w) -> p qc w", w=128)[:, q0:q1, :]
            rcp = rcpp.tile([128, 8], F32, name=f"rcp{id(pav)}_{p}_{q0}", tag="rcp")
            rcp3 = rcp.rearrange("p (a b) -> p a b", b=1)[:, 0:nq, :]
            nc.vector.reciprocal(rcp3, pavr[:, :, 0:1])
            if FUSED_NORM:
                rcp_b = bass.AP(tensor=rcp.tensor, offset=rcp.offset,
                                ap=[list(rcp.ap[0]), [1, nq], [0, 64]])
                nc.vector.tensor_mul(avn3[:, q0:q1, ho:ho + 64], pavr[:, :, 1:65], rcp_b)
            else:
                for qc in range(nq):
                    nc.vector.tensor_scalar_mul(
                        avn3[:, q0 + qc:q0 + qc + 1, ho:ho + 64],
                        pavr[:, qc:qc + 1, 1:65], rcp[:, qc:qc + 1])

        def emit_T(hc, qh, avn):
            aT3 = aT[hc].rearrange("p (tc t) -> p tc t", t=128)
            if TRANSPOSE_3D:
                nc.sync.dma_start_transpose(aT3[:, qh * 8:(qh + 1) * 8, :], avn)
            else:
                for tcn in range(8):
                    nc.sync.dma_start_transpose(
                        aT[hc][:, qh * 1024 + tcn * 128:qh * 1024 + (tcn + 1) * 128],
                        avn[:, tcn * 128:(tcn + 1) * 128])

        def emit_S_E(h, qh, kc, kt, qt):
            ho = (h % 2) * 64
            s = psS.tile([128, 1024], F32, name=f"s{h}_{qh}_{kc}", tag="s")
            ksl = slice(kc * 128, (kc + 1) * 128)
            for qq in range(2):
                qsl = slice(qh * 1024 + qq * 512, qh * 1024 + (qq + 1) * 512)
                nc.tensor.matmul(
                    s[:, qq * 512:(qq + 1) * 512],
                    kt[ho:ho + 64, ksl], qt[ho:ho + 64, qsl],
                    start=True, stop=True)
            pool = ex1p if (h, qh) == (1, 0) else expool
            tag = "ex1" if (h, qh) == (1, 0) else "ex"
            ex = pool.tile([128, 1024], BF16, name=f"ex{h}_{qh}_{kc}", tag=tag)
            nc.scalar.activation(ex, s, EXP, bias=0.0, scale=float(SCALE))
            return ex

        # ======== fused wall: units (h0,qh0)+(h1,qh0) share one S/E stream ==
        # All v-chunks and the remaining pair-0 qk slots run here as filler;
        # h1's AV is deferred into unit (h0,qh1) so the Activation engine gets
        # two units of exp supply while PE chews through the projection wall.
        emit_qk_slot(0, 0, psS)
        emit_qk_slot(4, 0, psS)
        emit_qk_slot(0, 1, psS)
        emit_rope_flush(psS)
        wall_slots = {0: (4, 1), 2: (4, 2), 4: (0, 2), 6: (4, 3), 8: (0, 3)}
        pav0 = psAV.tile([128, 1024], F32, name="pav0", tag="pav")
        ex0s = [None] * NKC
        ex1s = [None] * NKC
        avn00 = avnp.tile([128, 1024], BF16, name="avn00", tag="avn")
        avn00_3 = avn00.rearrange("p (tc w) -> p tc w", w=128)
        for kc in range(NKC):
            if kc in wall_slots:
                emit_qk_slot(*wall_slots[kc], psS)
            elif kc == 10:
                emit_rope_flush(psS)
            ex0s[kc] = emit_S_E(0, 0, kc, qk[4], qk[0])
            ex1s[kc] = emit_S_E(1, 0, kc, qk[4], qk[0])
            if kc <= 13:
                emit_v(kc, psF)
            if kc == 14:
                emit_v(14, psF)
                emit_v(15, psF)
            if kc >= 4:
                emit_av(pav0, ex0s[kc - 4], kc - 4, 0)
        for kc in range(NKC - 4, NKC):
            emit_av(pav0, ex0s[kc], kc, 0)
        dbg("dbg_ex0", ex0s[0])
        norm(pav0, 0, avn00_3)
        dbg("dbg_avn0", avn00)

        pav1_box = [None]

        def a1_item(lo, hi):
            def f():
                if pav1_box[0] is None:
                    pav1_box[0] = psAV.tile([128, 1024], F32, name="pav1", tag="pav")
                for kc2 in range(lo, hi):
                    emit_av(pav1_box[0], ex1s[kc2], kc2, 1)
            return f

        def norm1_item():
            norm(pav1_box[0], 1, avn00_3)
            emit_T(0, 0, avn00)

        # filler items per unit index (u = 4*hc + 2*qh + p)
        def qk_item(jc, tb):
            return lambda: emit_qk_slot(jc, tb, psF)

        flush = lambda: emit_rope_flush(psF)

        def qk_split(it, jc, tb):
            box = {}
            return [(it, lambda: emit_qk_part(jc, tb, psF, 0, box)),
                    (it + 1, lambda: emit_qk_part(jc, tb, psF, 1, box))]

        # per-unit fillers as (iteration, item) pairs; iterations chosen so
        # every RoPE combine (riding the next slot) lands >=2 iterations
        # before the first S that reads the roped chunk.
        unit_fill = {u: [] for u in range(16)}
        unit_fill[2] = [(0, a1_item(0, 8)), (2, a1_item(8, 16)), (4, norm1_item)] \
            + qk_split(8, 1, 0) + qk_split(12, 1, 1)
        unit_fill[3] = qk_split(0, 5, 0) + qk_split(5, 5, 1)
        unit_fill[4] = qk_split(0, 5, 2) + qk_split(4, 5, 3) + [(8, flush)] \
            + qk_split(12, 1, 2)
        unit_fill[5] = qk_split(0, 1, 3) + qk_split(4, 2, 0) \
            + qk_split(8, 2, 1) + qk_split(12, 6, 0)
        unit_fill[6] = qk_split(0, 2, 2) + qk_split(5, 6, 1) + qk_split(10, 2, 3)
        unit_fill[7] = qk_split(0, 6, 2) + qk_split(5, 6, 3) + [(10, flush)]
        unit_fill[8] = qk_split(0, 3, 0) + qk_split(5, 3, 1) + qk_split(10, 7, 0)
        unit_fill[9] = qk_split(0, 3, 2) + qk_split(5, 7, 1) + qk_split(10, 3, 3)
        unit_fill[10] = qk_split(0, 7, 2) + qk_split(5, 7, 3) + [(10, flush)]
        unit_fill[14] = [(2 * ec + 4, (lambda e: (lambda: emit_proj(e, 0, psF)))(ec))
                         for ec in range(8)]
        unit_fill[15] = [(2 * ec, (lambda e: (lambda: emit_proj(e, 1, psF)))(ec))
                         for ec in range(8)]

        # ---------------- remaining attention units ----------------
        for hc in range(4):
            for qh in range(2):
                if hc == 0 and qh == 0:
                    continue  # handled by the fused wall above
                avn = avnp.tile([128, 1024], BF16, name=f"avn{hc}_{qh}", tag="avn")
                avn3 = avn.rearrange("p (tc w) -> p tc w", w=128)
                for p in range(2):
                    h = 2 * hc + p
                    u = 4 * hc + 2 * qh + p
                    qt = qk[hc]
                    kt = qk[4 + hc]
                    fills = dict(unit_fill[u])
                    av_lag = 5
                    last_u = (hc, qh, p) == (3, 1, 1)
                    # pav is allocated at first use so psAV slot rotation
                    # follows emission order (pav1 is created inside u2's
                    # fillers, before this unit's first AV matmul).
                    pav = None
                    exs = [None] * NKC
                    ai = 0
                    for kc in range(NKC):
                        exs[kc] = emit_S_E(h, qh, kc, kt, qt)
                        if kc in fills:
                            fills.pop(kc)()
                        if kc >= av_lag:
                            if pav is None:
                                pav = psAV.tile([128, 1024], F32,
                                                name=f"pav{h}_{qh}", tag="pav")
                            # the final unit catches its deferred AVs up during
                            # its ACT-bound late iterations to shorten the tail
                            navs = 2 if (last_u and kc >= 12) else 1
                            for _ in range(navs):
                                if ai < kc:
                                    emit_av(pav, exs[ai], ai, h)
                                    ai += 1
                    for it in sorted(fills):
                        fills.pop(it)()
                    for kc in range(ai, NKC):
                        emit_av(pav, exs[kc], kc, h)
                    if (hc, qh, p) == (3, 1, 1):
                        # final unit: normalize + transpose in halves so the
                        # tail's th2 projection can start on the first half
                        norm(pav, p, avn3, 0, 4)
                        aT3f = aT[3].rearrange("p (tc t) -> p tc t", t=128)
                        nc.sync.dma_start_transpose(
                            aT3f[:, 8:12, :], avn[:, 0:512])
                        norm(pav, p, avn3, 4, 8)
                        nc.sync.dma_start_transpose(
                            aT3f[:, 12:16, :], avn[:, 512:1024])
                    else:
                        norm(pav, p, avn3)
                if (hc, qh) != (3, 1):
                    emit_T(hc, qh, avn)
        dbg("dbg_aT0", aT[0])

        # ---------------- tail: second half of projection ----------------
        # th2 prerun: open all 8 ec-groups and run their hcc 0-2 matmuls
        # while the last pair's normalize + transpose completes; the hcc3
        # matmul (stop) lands right after aT[3] arrives. Keeps PE busy with
        # no p-state reset across the transpose latency.
        tsl2 = slice(2 * 512, 3 * 512)
        tpools = [psS, psS, psF, psAV]
        t2 = [tpools[j].tile([128, 1024], F32, name=f"tt{j}",
                             tag="pav" if tpools[j] is psAV else "s")
              for j in range(4)]
        for j in range(4):
            for half in range(2):
                ec = 2 * j + half
                sl = slice(half * 512, (half + 1) * 512)
                for i in range(3):
                    nc.tensor.matmul(
                        t2[j][:, sl], wp_t[:, i * 1024 + ec * 128:i * 1024 + (ec + 1) * 128],
                        aT[i][:, tsl2], start=(i == 0), stop=False)
        for j in range(4):
            for half in range(2):
                ec = 2 * j + half
                sl = slice(half * 512, (half + 1) * 512)
                nc.tensor.matmul(
                    t2[j][:, sl], wp_t[:, 3 * 1024 + ec * 128:3 * 1024 + (ec + 1) * 128],
                    aT[3][:, tsl2], start=False, stop=True)
                osb = osbp.tile([128, 512], F32, name=f"osb{ec}_t2", tag="osb")
                nc.vector.tensor_scalar_add(osb, t2[j][:, sl], bpr_t[:, ec:ec + 1])
                nc.sync.dma_start(out[ec * 128:(ec + 1) * 128, tsl2], osb)
        th3_pools = [psS, psS, psF]
        for i, ec in enumerate(range(8)):
            emit_proj(ec, 3, th3_pools[i % 3])


def _input_specs():
    # name -> (shape, dtype_str)
    return {
        "xT": ((C, T), "bf16"), "wqkv": ((C, 3 * C // G), "bf16"),
        "wproj": ((C // G, C), "bf16"),
        "bqk": ((128, 8), "f32"), "bv": ((128, 520), "f32"),
        "bpr": ((128, 8), "f32"),
        "cosT": ((128, T), "bf16"), "sinT": ((128, T), "bf16"),
        "rope_perm": ((128, 128), "bf16"),
    }


def _build_program():
    import concourse.mybir as mybir
    import concourse.tile as tile
    from concourse import bacc

    nc = bacc.Bacc("TRN2", target_bir_lowering=False, debug=False)
    ins = {}
    for name, (shape, dts) in _input_specs().items():
        dt = mybir.dt.bfloat16 if dts == "bf16" else mybir.dt.float32
        ins[name] = nc.dram_tensor(name, list(shape), dt,
                                   kind="ExternalInput").ap()
    outs = {"out": nc.dram_tensor("out", [C, T], mybir.dt.float32,
                                  kind="ExternalOutput").ap()}
    with tile.TileContext(nc) as tc:
        _attn_body(tc, outs, ins)
    nc.compile()
    return nc


def _core_inputs(core, x, W_qkv, b_qkv, W_proj, b_proj, cosT, sinT, P):
    b, g = divmod(core, 2)
    f32 = np.float32
    bf16 = ml_dtypes.bfloat16
    xT = np.ascontiguousarray(np.asarray(x[b], dtype=f32).T).astype(bf16)
    W_qkv = np.asarray(W_qkv, dtype=f32)
    b_qkv = np.asarray(b_qkv, dtype=f32)
    q = W_qkv[:, g * 512:(g + 1) * 512]
    k = W_qkv[:, C + g * 512:C + (g + 1) * 512]
    v = W_qkv[:, 2 * C + g * 512:2 * C + (g + 1) * 512]
    wqkv = np.ascontiguousarray(np.concatenate([q, k, v], axis=1)).astype(bf16)
    bq = b_qkv[g * 512:(g + 1) * 512]
    bk = b_qkv[C + g * 512:C + (g + 1) * 512]
    bqk = np.ascontiguousarray(
        np.stack([bq[i * 128:(i + 1) * 128] for i in range(4)]
                 + [bk[i * 128:(i + 1) * 128] for i in range(4)], axis=1))
    bvr = b_qkv[2 * C + g * 512:2 * C + (g + 1) * 512].reshape(HG, 64)
    bvg = np.concatenate([np.ones((HG, 1), f32), bvr], axis=1).reshape(-1)  # (520,)
    bv = np.ascontiguousarray(np.tile(bvg[None, :], (128, 1)))
    wproj = np.ascontiguousarray(
        np.asarray(W_proj, dtype=f32)[g * 512:(g + 1) * 512]).astype(bf16)
    if g == 0:
        bpr = np.ascontiguousarray(
            np.asarray(b_proj, dtype=f32).reshape(8, 128).T)
    else:
        bpr = np.zeros((128, 8), dtype=f32)
    return {"xT": xT, "wqkv": wqkv, "wproj": wproj, "bqk": bqk, "bv": bv,
            "bpr": bpr, "cosT": cosT, "sinT": sinT, "rope_perm": P}


def run(x, W_qkv, b_qkv, W_proj, b_proj, trace=False):
    from concourse.bass_utils import run_bass_kernel_spmd

    if "nc" not in _CACHED:
        _CACHED["nc"] = _build_program()
    nc = _CACHED["nc"]

    bf16 = ml_dtypes.bfloat16
    cosT, sinT = _rope_tables()
    cosT = cosT.astype(bf16)
    sinT = sinT.astype(bf16)
    P = _perm_table().astype(bf16)
    in_maps = [_core_inputs(c, x, W_qkv, b_qkv, W_proj, b_proj, cosT, sinT, P)
               for c in range(8)]
    res = run_bass_kernel_spmd(nc, in_maps, core_ids=list(range(8)), trace=trace)
    parts = [np.asarray(r["out"], dtype=np.float32) for r in res.results]
    out = np.stack([(parts[2 * b] + parts[2 * b + 1]).T for b in range(B)], axis=0)
    return np.ascontiguousarray(out), res


def kernel(x, W_qkv, b_qkv, W_proj, b_proj):
    out, _ = run(x, W_qkv, b_qkv, W_proj, b_proj, trace=False)
    return out


# revision 57
# speedup vs baseline: 1.0022x; 1.0022x over previous
"""Multi-head attention (RoPE) Trainium2 Bass kernel — pipelined bf16 version.

Problem: B=4, T=2048, C=1024, H=16, d=64, fp32 in/out, full attention + RoPE.
Sharding: 8 cores = 4 batches x 2 head-groups (8 heads each). Each core
computes its batch's attention for its heads plus the partial (transposed)
output projection; the host sums the two head-group partials per batch and
transposes back.

Design notes (cost-model driven):
- All matmul operands are bf16 (1 cycle/row on PE, half the SBUF/DMA of f32).
- AV uses a token-major dataflow: out[q, 65] = ex_chunk^T @ [ones|v], using
  all 128 output partitions (halves AV PE time vs a 65-partition head-major
  form) and making softmax normalization a per-partition scalar multiply.
  The softmax denominator rides along as column 0 via the ones column of vg.
- Normalized attention output transposes back to head-dim-major via one
  SBUF->SBUF DMA xbar transpose per (head-pair, q-half); PE is not involved.
- One instruction stream software-pipelines everything: QKV chunk
  projections, v-projections and the second-half output projection run as PE
  filler inside the ACT-bound attention stretch so neither PE nor the
  Activation engine (exp) ever starves. RoPE for chunk i-1 is emitted inside
  chunk i's slot so its PE permutation-matmul never waits on DVE.
- Projection is emitted transposed (features on partitions) so its bias is a
  per-partition scalar; the host transposes the final result (untimed).
"""

import numpy as np
import ml_dtypes

B, T, C = 4, 2048, 1024
H, D = 16, 64
G = 2              # head groups (cores per batch)
HG = H // G        # heads per core = 8
CC = C // 128      # 8 contraction chunks
NKC = T // 128     # 16 key chunks
NTB = T // 512     # 4 t-blocks
ROPE_BASE = 10000.0
SCALE = 1.0 / np.sqrt(D)

FUSED_NORM = True      # stride-0 free-dim broadcast of 1/den in one DVE op
TRANSPOSE_3D = True    # one xbar DMA transpose per (pair, q-half)

_CACHED = {}


def _rope_tables():
    inv_freq = 1.0 / (ROPE_BASE ** (np.arange(0, D, 2, dtype=np.float32) / D))
    t = np.arange(T, dtype=np.float32)
    freqs = np.outer(t, inv_freq).astype(np.float32)          # (T, 32)
    emb = np.concatenate([freqs, freqs], axis=-1)             # (T, 64)
    cos = np.cos(emb).T.astype(np.float32)                    # (64, T)
    sin = np.sin(emb).T.astype(np.float32)                    # (64, T)
    cosT = np.concatenate([cos, cos], axis=0)                 # (128, T) two heads/chunk
    sinT = np.concatenate([sin, sin], axis=0)
    return np.ascontiguousarray(cosT), np.ascontiguousarray(sinT)


def _perm_table():
    # rot[d] = sum_s P[s, d] * raw[s] = rotate_half with sign, 2 heads/chunk
    P = np.zeros((128, 128), np.float32)
    for d in range(128):
        blk, dd = divmod(d, D)
        if dd < 32:
            P[blk * D + dd + 32, d] = -1.0
        else:
            P[blk * D + dd - 32, d] = 1.0
    return P


def _attn_body(tc, outs, ins):
    """Tile kernel body. ins/outs are dicts of DRAM APs."""
    import contextlib
    import concourse.bass as bass
    import concourse.mybir as mybir

    nc = tc.nc
    F32 = mybir.dt.float32
    BF16 = mybir.dt.bfloat16
    EXP = mybir.ActivationFunctionType.Exp

    xT = ins["xT"]            # (1024, 2048) bf16  x[b].T
    wqkv = ins["wqkv"]        # (1024, 1536) bf16  [Wq | Wk | Wv] cols for group
    wproj = ins["wproj"]      # (512, 1024) bf16
    bqk = ins["bqk"]          # (128, 8) f32 per-chunk per-partition bias
    bv = ins["bv"]            # (128, 520) f32 broadcast [1|v-bias] per head
    bpr = ins["bpr"]          # (128, 8) f32 proj bias (e-chunk cols; zeros g1)
    cosT_d = ins["cosT"]      # (128, 2048) bf16
    sinT_d = ins["sinT"]      # (128, 2048) bf16
    perm_d = ins["rope_perm"]  # (128, 128) bf16 signed rotate_half permutation
    out = outs["out"]         # (1024, 2048) f32 partial transposed output

    def dbg(name, tile_ap):
        if name in outs:
            nc.sync.dma_start(outs[name].bitcast(tile_ap.dtype), tile_ap)

    ctx = contextlib.ExitStack()
    with ctx:
        pers = ctx.enter_context(tc.tile_pool(name="pers", bufs=1))

        # ---------------- persistent tiles ----------------
        x_t = pers.tile([128, CC * T], BF16, name="x_t", tag="x_t")
        wqk_t = pers.tile([128, CC * 1024], BF16, name="wqk_t", tag="wqk_t")
        wv_t = pers.tile([128, CC * 512], BF16, name="wv_t", tag="wv_t")
        wp_t = pers.tile([128, 4 * 1024], BF16, name="wp_t", tag="wp_t")
        cos_t = pers.tile([128, T], BF16, name="cos_t", tag="cos_t")
        sin_t = pers.tile([128, T], BF16, name="sin_t", tag="sin_t")
        perm_t = pers.tile([128, 128], BF16, name="perm_t", tag="perm_t")
        bqk_t = pers.tile([128, 8], F32, name="bqk_t", tag="bqk_t")
        bv_t = pers.tile([128, 520], F32, name="bv_t", tag="bv_t")
        bpr_t = pers.tile([128, 8], F32, name="bpr_t", tag="bpr_t")
        qk = [pers.tile([128, T], BF16, name=f"qk{j}", tag=f"qk{j}") for j in range(8)]
        vg = [pers.tile([128, HG * 65], BF16, name=f"vg{k}", tag=f"vg{k}") for k in range(NKC)]
        aT = [pers.tile([128, T], BF16, name=f"aT{i}", tag=f"aT{i}") for i in range(4)]

        # ---------------- working pools ----------------
        # PSUM: psS 2x2 banks (scores/exp), psAV 1x2 banks (AV accum),
        # psF 1x2 banks (qkv/v/proj filler groups + rope perm outputs).
        psS = ctx.enter_context(tc.tile_pool(name="psS", bufs=2, space="PSUM"))
        psAV = ctx.enter_context(tc.tile_pool(name="psAV", bufs=1, space="PSUM"))
        psF = ctx.enter_context(tc.tile_pool(name="psF", bufs=1, space="PSUM"))
        expool = ctx.enter_context(tc.tile_pool(name="expool", bufs=6))
        ex1p = ctx.enter_context(tc.tile_pool(name="ex1p", bufs=16))
        rawp = ctx.enter_context(tc.tile_pool(name="rawp", bufs=2))
        tmpp = ctx.enter_context(tc.tile_pool(name="tmpp", bufs=2))
        tmpcp = ctx.enter_context(tc.tile_pool(name="tmpcp", bufs=2))
        denp = ctx.enter_context(tc.tile_pool(name="denp", bufs=2))
        rcpp = ctx.enter_context(tc.tile_pool(name="rcpp", bufs=2))
        avnp = ctx.enter_context(tc.tile_pool(name="avnp", bufs=2))
        osbp = ctx.enter_context(tc.tile_pool(name="osbp", bufs=5))

        # ---------------- input DMAs (no waits; ordered for earliest use) ---
        x3d = x_t.rearrange("p (c t) -> p c t", t=T)
        xTd = xT.rearrange("(c p) t -> p c t", p=128)
        wqk3 = wqk_t.rearrange("p (c e) -> p c e", e=1024)

        def w_slice(jc):
            # per-chunk 128-col slice of [Wq|Wk] for q/k chunk jc
            col0 = (jc % 4) * 128 + (512 if jc >= 4 else 0)
            nc.sync.dma_start(
                wqk3[:, :, col0:col0 + 128],
                wqkv[:, col0:col0 + 128].rearrange("(c p) e -> p c e", p=128))

        nc.sync.dma_start(perm_t, perm_d)
        w_slice(0)
        nc.sync.dma_start(x3d[:, :, 0:512], xTd[:, :, 0:512])
        nc.sync.dma_start(bqk_t, bqk)
        nc.sync.dma_start(cos_t[:, 0:1024], cosT_d[:, 0:1024])
        nc.sync.dma_start(sin_t[:, 0:1024], sinT_d[:, 0:1024])
        w_slice(4)
        nc.sync.dma_start(x3d[:, :, 512:1024], xTd[:, :, 512:1024])
        nc.sync.dma_start(cos_t[:, 1024:2048], cosT_d[:, 1024:2048])
        nc.sync.dma_start(sin_t[:, 1024:2048], sinT_d[:, 1024:2048])
        nc.sync.dma_start(wv_t.rearrange("p (c e) -> p c e", e=512),
                          wqkv[:, 1024:1536].rearrange("(c p) e -> p c e", p=128))
        nc.sync.dma_start(bv_t, bv)
        nc.sync.dma_start(x3d[:, :, 1024:1536], xTd[:, :, 1024:1536])
        nc.sync.dma_start(x3d[:, :, 1536:2048], xTd[:, :, 1536:2048])
        for jc in (1, 5, 2, 6, 3, 7):
            w_slice(jc)
        nc.sync.dma_start(bpr_t, bpr)
        nc.sync.dma_start(wp_t.rearrange("p (i e) -> p i e", e=1024),
                          wproj.rearrange("(i p) e -> p i e", p=128))

        # PE clock warmup: keep the tensor engine continuously busy from the
        # moment perm_t lands until the first real slot's inputs arrive, so
        # the p-state model reaches full clock before real work dispatches.
        warm = psF.tile([128, 1024], F32, name="warm", tag="s")
        for i in range(40):
            nc.tensor.matmul(warm[:, 0:128], perm_t, perm_t, start=True, stop=True)

        # ---------------- emitters ----------------
        pend = {"rope": None}  # (jc, tb, raw) awaiting perm-matmul + combine

        def emit_rope(ps_half):
            """Emit pending RoPE combine: perm-matmul into ps_half (psum
            (128,512) f32 slice), then DVE combine into qk[jc]."""
            jc, tb, raw = pend["rope"]
            pend["rope"] = None
            tsl = slice(tb * 512, (tb + 1) * 512)
            nc.tensor.matmul(ps_half, perm_t, raw, start=True, stop=True)
            tmp = tmpp.tile([128, 512], F32, name=f"tm{jc}_{tb}", tag="tmp")
            nc.vector.tensor_mul(tmp, ps_half, sin_t[:, tsl])
            tmpc = tmpcp.tile([128, 512], F32, name=f"tc{jc}_{tb}", tag="tmpc")
            nc.vector.tensor_mul(tmpc, raw, cos_t[:, tsl])
            nc.vector.tensor_add(qk[jc][:, tsl], tmp, tmpc)

        def emit_qk_part(jc, tb, pool, part, box):
            """Half of a q/k chunk slot: part 0 allocs the psum tile, emits
            the pending RoPE combine and contraction chunks 0-3; part 1 emits
            chunks 4-7 and the bias. Splitting keeps each filler burst under
            the exp pipeline's slack."""
            col0 = (jc % 4) * 128 + (512 if jc >= 4 else 0)
            if part == 0:
                ps = pool.tile([128, 1024], F32, name=f"psq{jc}_{tb}", tag="s")
                box["ps"] = ps
                if pend["rope"] is not None:
                    emit_rope(ps[:, 512:1024])
                crange = range(0, CC // 2)
            else:
                ps = box["ps"]
                crange = range(CC // 2, CC)
            for c in crange:
                nc.tensor.matmul(
                    ps[:, 0:512], wqk_t[:, c * 1024 + col0:c * 1024 + col0 + 128],
                    x_t[:, c * T + tb * 512:c * T + (tb + 1) * 512],
                    start=(c == 0), stop=(c == CC - 1))
            if part == 1:
                raw = rawp.tile([128, 512], BF16, name=f"raw{jc}_{tb}", tag="raw")
                nc.vector.tensor_scalar_add(raw, ps[:, 0:512], bqk_t[:, jc:jc + 1])
                pend["rope"] = (jc, tb, raw)

        def emit_qk_slot(jc, tb, pool):
            """8 projection matmuls for q/k chunk jc, t-block tb, plus the
            RoPE combine of the previously emitted chunk."""
            box = {}
            emit_qk_part(jc, tb, pool, 0, box)
            emit_qk_part(jc, tb, pool, 1, box)

        def emit_rope_flush(pool):
            ps = pool.tile([128, 1024], F32, name="psflush", tag="s")
            emit_rope(ps[:, 512:1024])

        def emit_v(kc, pool):
            """v for token chunk kc -> vg[kc] = [1|v] per head, bf16."""
            ps = pool.tile([128, 1024], F32, name=f"psv{kc}", tag="s")
            for c in range(CC):
                nc.tensor.matmul(
                    ps[:, 0:512], x_t[:, c * T + kc * 128:c * T + (kc + 1) * 128],
                    wv_t[:, c * 512:(c + 1) * 512],
                    start=(c == 0), stop=(c == CC - 1))
            vv = vg[kc].rearrange("p (g w) -> p g w", w=65)
            bvv = bv_t.rearrange("p (g w) -> p g w", w=65)
            psg = ps[:, 0:512].rearrange("p (g d) -> p g d", d=64)
            nc.vector.tensor_add(vv[:, :, 1:65], psg, bvv[:, :, 1:65])
            nc.vector.tensor_copy(vv[:, :, 0:1], bvv[:, :, 0:1])

        IDENT = mybir.ActivationFunctionType.Identity

        def emit_bias(osb, ps_slice, ec, on_act):
            # tail biases run on the idle Activation engine (Identity with a
            # per-partition AP bias = add + psum->sbuf copy in one op)
            if on_act:
                nc.scalar.activation(osb, ps_slice, IDENT,
                                     bias=bpr_t[:, ec:ec + 1], scale=1.0)
            else:
                nc.vector.tensor_scalar_add(osb, ps_slice, bpr_t[:, ec:ec + 1])

        def emit_proj(ec, th, pool, on_act=False):
            """transposed proj: out rows = e-chunk ec, cols = tok block th."""
            tsl = slice(th * 512, (th + 1) * 512)
            ps = pool.tile([128, 1024], F32, name=f"psp{ec}_{th}", tag="s")
            for i in range(4):
                nc.tensor.matmul(
                    ps[:, 0:512], wp_t[:, i * 1024 + ec * 128:i * 1024 + (ec + 1) * 128],
                    aT[i][:, tsl], start=(i == 0), stop=(i == 3))
            osb = osbp.tile([128, 512], F32, name=f"osb{ec}_{th}", tag="osb")
            emit_bias(osb, ps[:, 0:512], ec, on_act)
            nc.sync.dma_start(out[ec * 128:(ec + 1) * 128, tsl], osb)

        def emit_av(pav, ex, kc, h):
            # start=True zeroes the whole PSUM bank, so only the first group
            # of each bank (qc 0 and 4) may set it; the bank-wide zero covers
            # the other interleaved accumulation groups' regions.
            mv = vg[kc][:, h * 65:(h + 1) * 65]
            for qc in range(8):
                nc.tensor.matmul(
                    pav[:, qc * 128:qc * 128 + 65],
                    ex[:, qc * 128:(qc + 1) * 128], mv,
                    start=(kc == 0 and qc % 4 == 0), stop=(kc == NKC - 1))

        def norm(pav, p, avn3, q0=0, q1=8):
            """normalize: avn[:, tc, p*64+d] = pav[:, tc, 1+d] / pav[:, tc, 0]"""
            ho = p * 64
            nq = q1 - q0
            pavr = pav.rearrange("p (qc w) -> p qc w", w=128)[:, q0:q1, :]
            rcp = rcpp.tile([128, 8], F32, name=f"rcp{id(pav)}_{p}_{q0}", tag="rcp")
            rcp3 = rcp.rearrange("p (a b) -> p a b", b=1)[:, 0:nq, :]
            nc.vector.reciprocal(rcp3, pavr[:, :, 0:1])
            if FUSED_NORM:
                rcp_b = bass.AP(tensor=rcp.tensor, offset=rcp.offset,
                                ap=[list(rcp.ap[0]), [1, nq], [0, 64]])
                nc.vector.tensor_mul(avn3[:, q0:q1, ho:ho + 64], pavr[:, :, 1:65], rcp_b)
            else:
                for qc in range(nq):
                    nc.vector.tensor_scalar_mul(
                        avn3[:, q0 + qc:q0 + qc + 1, ho:ho + 64],
                        pavr[:, qc:qc + 1, 1:65], rcp[:, qc:qc + 1])

        def emit_T(hc, qh, avn):
            aT3 = aT[hc].rearrange("p (tc t) -> p tc t", t=128)
            if TRANSPOSE_3D:
                nc.sync.dma_start_transpose(aT3[:, qh * 8:(qh + 1) * 8, :], avn)
            else:
                for tcn in range(8):
                    nc.sync.dma_start_transpose(
                        aT[hc][:, qh * 1024 + tcn * 128:qh * 1024 + (tcn + 1) * 128],
                        avn[:, tcn * 128:(tcn + 1) * 128])

        def emit_S_E(h, qh, kc, kt, qt):
            ho = (h % 2) * 64
            s = psS.tile([128, 1024], F32, name=f"s{h}_{qh}_{kc}", tag="s")
            ksl = slice(kc * 128, (kc + 1) * 128)
            for qq in range(2):
                qsl = slice(qh * 1024 + qq * 512, qh * 1024 + (qq + 1) * 512)
                nc.tensor.matmul(
                    s[:, qq * 512:(qq + 1) * 512],
                    kt[ho:ho + 64, ksl], qt[ho:ho + 64, qsl],
                    start=True, stop=True)
            pool = ex1p if (h, qh) == (1, 0) else expool
            tag = "ex1" if (h, qh) == (1, 0) else "ex"
            ex = pool.tile([128, 1024], BF16, name=f"ex{h}_{qh}_{kc}", tag=tag)
            nc.scalar.activation(ex, s, EXP, bias=0.0, scale=float(SCALE))
            return ex

        # ======== fused wall: units (h0,qh0)+(h1,qh0) share one S/E stream ==
        # All v-chunks and the remaining pair-0 qk slots run here as filler;
        # h1's AV is deferred into unit (h0,qh1) so the Activation engine gets
        # two units of exp supply while PE chews through the projection wall.
        emit_qk_slot(0, 0, psS)
        emit_qk_slot(4, 0, psS)
        emit_qk_slot(0, 1, psS)
        emit_rope_flush(psS)
        wall_slots = {0: (4, 1), 2: (4, 2), 4: (0, 2), 6: (4, 3), 8: (0, 3)}
        pav0 = psAV.tile([128, 1024], F32, name="pav0", tag="pav")
        ex0s = [None] * NKC
        ex1s = [None] * NKC
        avn00 = avnp.tile([128, 1024], BF16, name="avn00", tag="avn")
        avn00_3 = avn00.rearrange("p (tc w) -> p tc w", w=128)
        for kc in range(NKC):
            if kc in wall_slots:
                emit_qk_slot(*wall_slots[kc], psS)
            elif kc == 10:
                emit_rope_flush(psS)
            ex0s[kc] = emit_S_E(0, 0, kc, qk[4], qk[0])
            ex1s[kc] = emit_S_E(1, 0, kc, qk[4], qk[0])
            if kc <= 13:
                emit_v(kc, psF)
            if kc == 14:
                emit_v(14, psF)
                emit_v(15, psF)
            if kc >= 4:
                emit_av(pav0, ex0s[kc - 4], kc - 4, 0)
        for kc in range(NKC - 4, NKC):
            emit_av(pav0, ex0s[kc], kc, 0)
        dbg("dbg_ex0", ex0s[0])
        norm(pav0, 0, avn00_3)
        dbg("dbg_avn0", avn00)

        pav1_box = [None]

        def a1_item(lo, hi):
            def f():
                if pav1_box[0] is None:
                    pav1_box[0] = psAV.tile([128, 1024], F32, name="pav1", tag="pav")
                for kc2 in range(lo, hi):
                    emit_av(pav1_box[0], ex1s[kc2], kc2, 1)
            return f

        def norm1_item():
            norm(pav1_box[0], 1, avn00_3)
            emit_T(0, 0, avn00)

        # filler items per unit index (u = 4*hc + 2*qh + p)
        def qk_item(jc, tb):
            return lambda: emit_qk_slot(jc, tb, psF)

        flush = lambda: emit_rope_flush(psF)

        def qk_split(it, jc, tb):
            box = {}
            return [(it, lambda: emit_qk_part(jc, tb, psF, 0, box)),
                    (it + 1, lambda: emit_qk_part(jc, tb, psF, 1, box))]

        # per-unit fillers as (iteration, item) pairs; iterations chosen so
        # every RoPE combine (riding the next slot) lands >=2 iterations
        # before the first S that reads the roped chunk.
        unit_fill = {u: [] for u in range(16)}
        unit_fill[2] = [(0, a1_item(0, 8)), (2, a1_item(8, 16)), (4, norm1_item)] \
            + qk_split(8, 1, 0) + qk_split(12, 1, 1)
        unit_fill[3] = qk_split(0, 5, 0) + qk_split(5, 5, 1)
        unit_fill[4] = qk_split(0, 5, 2) + qk_split(4, 5, 3) + [(8, flush)] \
            + qk_split(12, 1, 2)
        unit_fill[5] = qk_split(0, 1, 3) + qk_split(4, 2, 0) \
            + qk_split(8, 2, 1) + qk_split(12, 6, 0)
        unit_fill[6] = qk_split(0, 2, 2) + qk_split(5, 6, 1) + qk_split(10, 2, 3)
        unit_fill[7] = qk_split(0, 6, 2) + qk_split(5, 6, 3) + [(10, flush)]
        unit_fill[8] = qk_split(0, 3, 0) + qk_split(5, 3, 1) + qk_split(10, 7, 0)
        unit_fill[9] = qk_split(0, 3, 2) + qk_split(5, 7, 1) + qk_split(10, 3, 3)
        unit_fill[10] = qk_split(0, 7, 2) + qk_split(5, 7, 3) + [(10, flush)]
        unit_fill[14] = [(2 * ec + 4, (lambda e: (lambda: emit_proj(e, 0, psF)))(ec))
                         for ec in range(8)]
        unit_fill[15] = [(2 * ec, (lambda e: (lambda: emit_proj(e, 1, psF)))(ec))
                         for ec in range(8)]

        # ---------------- remaining attention units ----------------
        for hc in range(4):
            for qh in range(2):
                if hc == 0 and qh == 0:
                    continue  # handled by the fused wall above
                avn = avnp.tile([128, 1024], BF16, name=f"avn{hc}_{qh}", tag="avn")
                avn3 = avn.rearrange("p (tc w) -> p tc w", w=128)
                for p in range(2):
                    h = 2 * hc + p
                    u = 4 * hc + 2 * qh + p
                    qt = qk[hc]
                    kt = qk[4 + hc]
                    fills = dict(unit_fill[u])
                    av_lag = 5
                    last_u = (hc, qh, p) == (3, 1, 1)
                    # pav is allocated at first use so psAV slot rotation
                    # follows emission order (pav1 is created inside u2's
                    # fillers, before this unit's first AV matmul).
                    pav = None
                    exs = [None] * NKC
                    ai = 0
                    for kc in range(NKC):
                        exs[kc] = emit_S_E(h, qh, kc, kt, qt)
                        if kc in fills:
                            fills.pop(kc)()
                        if kc >= av_lag:
                            if pav is None:
                                pav = psAV.tile([128, 1024], F32,
                                                name=f"pav{h}_{qh}", tag="pav")
                            # the final unit catches its deferred AVs up during
                            # its ACT-bound late iterations to shorten the tail
                            navs = 2 if (last_u and kc >= 12) else 1
                            for _ in range(navs):
                                if ai < kc:
                                    emit_av(pav, exs[ai], ai, h)
                                    ai += 1
                    for it in sorted(fills):
                        fills.pop(it)()
                    for kc in range(ai, NKC):
                        emit_av(pav, exs[kc], kc, h)
                    if (hc, qh, p) == (3, 1, 1):
                        # final unit: normalize + transpose in halves so the
                        # tail's th2 projection can start on the first half
                        norm(pav, p, avn3, 0, 4)
                        aT3f = aT[3].rearrange("p (tc t) -> p tc t", t=128)
                        nc.sync.dma_start_transpose(
                            aT3f[:, 8:12, :], avn[:, 0:512])
                        norm(pav, p, avn3, 4, 8)
                        nc.sync.dma_start_transpose(
                            aT3f[:, 12:16, :], avn[:, 512:1024])
                    else:
                        norm(pav, p, avn3)
                if (hc, qh) != (3, 1):
                    emit_T(hc, qh, avn)
        dbg("dbg_aT0", aT[0])

        # ---------------- tail: second half of projection ----------------
        # th2 prerun: open all 8 ec-groups and run their hcc 0-2 matmuls
        # while the last pair's normalize + transpose completes; the hcc3
        # matmul (stop) lands right after aT[3] arrives. Keeps PE busy with
        # no p-state reset across the transpose latency.
        tsl2 = slice(2 * 512, 3 * 512)
        tpools = [psS, psS, psF, psAV]
        t2 = [tpools[j].tile([128, 1024], F32, name=f"tt{j}",
                             tag="pav" if tpools[j] is psAV else "s")
              for j in range(4)]
        for j in range(4):
            for half in range(2):
                ec = 2 * j + half
                sl = slice(half * 512, (half + 1) * 512)
                for i in range(3):
                    nc.tensor.matmul(
                        t2[j][:, sl], wp_t[:, i * 1024 + ec * 128:i * 1024 + (ec + 1) * 128],
                        aT[i][:, tsl2], start=(i == 0), stop=False)
        for j in range(4):
            for half in range(2):
                ec = 2 * j + half
                sl = slice(half * 512, (half + 1) * 512)
                nc.tensor.matmul(
                    t2[j][:, sl], wp_t[:, 3 * 1024 + ec * 128:3 * 1024 + (ec + 1) * 128],
                    aT[3][:, tsl2], start=False, stop=True)
                osb = osbp.tile([128, 512], F32, name=f"osb{ec}_t2", tag="osb")
                emit_bias(osb, t2[j][:, sl], ec, on_act=bool(ec % 2))
                nc.sync.dma_start(out[ec * 128:(ec + 1) * 128, tsl2], osb)
        th3_pools = [psS, psS, psF]
        for i, ec in enumerate(range(8)):
            emit_proj(ec, 3, th3_pools[i % 3], on_act=bool(ec % 2))


def _input_specs():
    # name -> (shape, dtype_str)
    return {
        "xT": ((C, T), "bf16"), "wqkv": ((C, 3 * C // G), "bf16"),
        "wproj": ((C // G, C), "bf16"),
        "bqk": ((128, 8), "f32"), "bv": ((128, 520), "f32"),
        "bpr": ((128, 8), "f32"),
        "cosT": ((128, T), "bf16"), "sinT": ((128, T), "bf16"),
        "rope_perm": ((128, 128), "bf16"),
    }


def _build_program():
    import concourse.mybir as mybir
    import concourse.tile as tile
    from concourse import bacc

    nc = bacc.Bacc("TRN2", target_bir_lowering=False, debug=False)
    ins = {}
    for name, (shape, dts) in _input_specs().items():
        dt = mybir.dt.bfloat16 if dts == "bf16" else mybir.dt.float32
        ins[name] = nc.dram_tensor(name, list(shape), dt,
                                   kind="ExternalInput").ap()
    outs = {"out": nc.dram_tensor("out", [C, T], mybir.dt.float32,
                                  kind="ExternalOutput").ap()}
    with tile.TileContext(nc) as tc:
        _attn_body(tc, outs, ins)
    nc.compile()
    return nc


def _core_inputs(core, x, W_qkv, b_qkv, W_proj, b_proj, cosT, sinT, P):
    b, g = divmod(core, 2)
    f32 = np.float32
    bf16 = ml_dtypes.bfloat16
    xT = np.ascontiguousarray(np.asarray(x[b], dtype=f32).T).astype(bf16)
    W_qkv = np.asarray(W_qkv, dtype=f32)
    b_qkv = np.asarray(b_qkv, dtype=f32)
    q = W_qkv[:, g * 512:(g + 1) * 512]
    k = W_qkv[:, C + g * 512:C + (g + 1) * 512]
    v = W_qkv[:, 2 * C + g * 512:2 * C + (g + 1) * 512]
    wqkv = np.ascontiguousarray(np.concatenate([q, k, v], axis=1)).astype(bf16)
    bq = b_qkv[g * 512:(g + 1) * 512]
    bk = b_qkv[C + g * 512:C + (g + 1) * 512]
    bqk = np.ascontiguousarray(
        np.stack([bq[i * 128:(i + 1) * 128] for i in range(4)]
                 + [bk[i * 128:(i + 1) * 128] for i in range(4)], axis=1))
    bvr = b_qkv[2 * C + g * 512:2 * C + (g + 1) * 512].reshape(HG, 64)
    bvg = np.concatenate([np.ones((HG, 1), f32), bvr], axis=1).reshape(-1)  # (520,)
    bv = np.ascontiguousarray(np.tile(bvg[None, :], (128, 1)))
    wproj = np.ascontiguousarray(
        np.asarray(W_proj, dtype=f32)[g * 512:(g + 1) * 512]).astype(bf16)
    if g == 0:
        bpr = np.ascontiguousarray(
            np.asarray(b_proj, dtype=f32).reshape(8, 128).T)
    else:
        bpr = np.zeros((128, 8), dtype=f32)
    return {"xT": xT, "wqkv": wqkv, "wproj": wproj, "bqk": bqk, "bv": bv,
            "bpr": bpr, "cosT": cosT, "sinT": sinT, "rope_perm": P}


def run(x, W_qkv, b_qkv, W_proj, b_proj, trace=False):
    from concourse.bass_utils import run_bass_kernel_spmd

    if "nc" not in _CACHED:
        _CACHED["nc"] = _build_program()
    nc = _CACHED["nc"]

    bf16 = ml_dtypes.bfloat16
    cosT, sinT = _rope_tables()
    cosT = cosT.astype(bf16)
    sinT = sinT.astype(bf16)
    P = _perm_table().astype(bf16)
    in_maps = [_core_inputs(c, x, W_qkv, b_qkv, W_proj, b_proj, cosT, sinT, P)
               for c in range(8)]
    res = run_bass_kernel_spmd(nc, in_maps, core_ids=list(range(8)), trace=trace)
    parts = [np.asarray(r["out"], dtype=np.float32) for r in res.results]
    out = np.stack([(parts[2 * b] + parts[2 * b + 1]).T for b in range(B)], axis=0)
    return np.ascontiguousarray(out), res


def kernel(x, W_qkv, b_qkv, W_proj, b_proj):
    out, _ = run(x, W_qkv, b_qkv, W_proj, b_proj, trace=False)
    return out
